# revision 7
# baseline (speedup 1.0000x reference)
"""AdaptiveBarlowTwinsLoss on 8 TRN2 NeuronCores.

Math: with O = head_outputs reshaped (N, H*dh), standardized O~ = (O-mu)/(sigma+eps),
the loss only needs the 120 upper-triangular head-pair blocks of C = O~^T O~ / N.
Writing G = O^T O (raw gram), C[id, je] = G[id,je]*r[id]*r[je] - q[id]*q[je]
with r = 1/(sqrt(N)(sigma+eps)), q = sqrt(N)*mu*r.  So each core computes the raw
gram over its token shard (bf16 matmuls, fp32 PSUM accum), applies the affine
standardization correction locally (scaled by 1/8 for the rank-1 term), and a
ReduceScatter sums the corrected blocks while scattering 15 pairs to each core.
Each core then computes ||C_ij - I||_F^2 for its pairs; the host applies the
(16,16) softplus pair weights and averages.

Pair p (lexicographic (i,j), i<j) is assigned to core p % 8, slot p // 8.
"""

import math
import sys

sys.path.insert(0, "/opt/trn_rl_repo")

import numpy as np

import concourse.bass as bass
import concourse.tile as tile
from concourse import bacc, mybir
from concourse.bass_utils import run_bass_kernel_spmd
from concourse.masks import make_identity

B, T, H, DH = 8, 2048, 16, 128
N = B * T                      # 16384 tokens
F = H * DH                     # 2048 features
NC = 8                         # cores
NS = N // NC                   # 2048 tokens per core
KCH = NS // 128                # 16 contraction chunks of 128 tokens
ALPHA, BETA, TAU, EPS = 0.929, 15.99, 0.0, 1e-8

PAIRS = [(i, j) for i in range(H) for j in range(i + 1, H)]   # 120, lex order
NGEN = len(PAIRS) // 8                                        # 15 gens of 8 blocks
RS_SLOTS = [(0, 4), (4, 8), (8, 12), (12, 15)]                # 4 ReduceScatter calls

F32 = mybir.dt.float32
BF16 = mybir.dt.bfloat16


def build():
    nc = bacc.Bacc("TRN2", target_bir_lowering=False, debug=False, num_devices=NC)

    x = nc.dram_tensor("x", [NS, F], F32, kind="ExternalInput")
    out = nc.dram_tensor("out", [1, NGEN], F32, kind="ExternalOutput")
    groups = [list(range(NC))]

    with tile.TileContext(nc) as tc:
        with (
            tc.tile_pool(name="dram", bufs=1, space="DRAM") as dram,
            tc.tile_pool(name="xf", bufs=3) as xfp,
            tc.tile_pool(name="xb", bufs=1) as xbp,
            tc.tile_pool(name="gsb", bufs=6) as gsbp,
            tc.tile_pool(name="ps", bufs=3, space="PSUM") as psp,
            tc.tile_pool(name="sg", bufs=1) as sg,
            tc.tile_pool(name="post", bufs=2) as postp,
        ):
            # ---- DRAM internals ----
            stats_in = dram.tile([2, F], F32, tag="stats_in")
            stats_out = dram.tile([2, F], F32, tag="stats_out")
            rq_stage = dram.tile([2, F], F32, tag="rq_stage")
            bounce = [
                dram.tile([NC, hi - lo, DH, DH], F32, tag=f"bounce{t}", name=f"bounce{t}")
                for t, (lo, hi) in enumerate(RS_SLOTS)
            ]
            rsout = [
                dram.tile([hi - lo, DH, DH], F32, tag=f"rsout{t}", name=f"rsout{t}")
                for t, (lo, hi) in enumerate(RS_SLOTS)
            ]

            # ---- persistent SBUF ----
            xb = [
                xbp.tile([128, F], BF16, tag=f"xb{k}", name=f"xb{k}")
                for k in range(KCH)
            ]
            s1acc = sg.tile([128, F], F32, tag="s1acc")
            IT16 = sg.tile([128, F], F32, tag="it16")     # identity per head block
            IT = sg.tile([128, DH], F32, tag="it")
            onesf = sg.tile([128, 1], F32, tag="ones")
            R2 = sg.tile([128, F], F32, tag="r2")         # r broadcast down partitions
            Q2 = sg.tile([128, F], F32, tag="q2")         # q broadcast down partitions
            pl_cols = sg.tile([128, NGEN], F32, tag="plc")

            def colt(tag):
                return sg.tile([128, H], F32, tag=tag, name=tag)

            S1c, S2c = colt("s1c"), colt("s2c")
            mu, m2, var = colt("mu"), colt("m2"), colt("var")
            sig, recip = colt("sig"), colt("recip")
            r_col, q_col, q8_col = colt("rcol"), colt("qcol"), colt("q8col")

            # ---- constants ----
            nc.gpsimd.memset(onesf[:], 1.0)
            make_identity(nc, IT[:])
            nc.gpsimd.memset(IT16[:], 0.0)
            it3 = IT16[:].rearrange("p (i e) -> p i e", i=H)
            nc.gpsimd.affine_select(
                out=it3,
                in_=it3,
                compare_op=mybir.AluOpType.not_equal,
                fill=1.0,
                base=0,
                pattern=[[0, H], [-1, DH]],
                channel_multiplier=1,
            )

            # ---- load, cast to bf16, accumulate S1 ----
            for k in range(KCH):
                xf = xfp.tile([128, F], F32, tag="xf")
                nc.sync.dma_start(out=xf[:], in_=x[k * 128:(k + 1) * 128, :])
                nc.scalar.copy(out=xb[k][:], in_=xf[:])
                if k == 0:
                    nc.vector.tensor_copy(out=s1acc[:], in_=xf[:])
                else:
                    nc.vector.tensor_add(out=s1acc[:], in0=s1acc[:], in1=xf[:])

            # S1 partition reduce via ones-matmul -> s1row [1, F]
            s1row = sg.tile([1, F], F32, tag="s1row")
            for t4 in range(4):
                pss1 = psp.tile([1, 512], F32, tag="pss1", bufs=2)
                nc.tensor.matmul(
                    pss1[0:1, :],
                    lhsT=onesf[:],
                    rhs=s1acc[:, t4 * 512:(t4 + 1) * 512],
                    start=True,
                    stop=True,
                )
                nc.vector.tensor_copy(
                    out=s1row[0:1, t4 * 512:(t4 + 1) * 512], in_=pss1[0:1, :]
                )

            # ---- diagonal blocks G_ii (for S2 = diag) : 2 gens of 8 ----
            for dg in range(2):
                ps = psp.tile([128, 1024], F32, tag="ps")
                for k in range(KCH):
                    for b in range(8):
                        i = dg * 8 + b
                        sl = xb[k][:, i * DH:(i + 1) * DH]
                        nc.tensor.matmul(
                            ps[:, b * DH:(b + 1) * DH],
                            lhsT=sl,
                            rhs=sl,
                            start=(k == 0),
                            stop=(k == KCH - 1),
                        )
                gd = postp.tile([128, 1024], F32, tag="gd")
                nc.scalar.copy(out=gd[:], in_=ps[:])
                # extract diagonals: multiply by tiled identity, reduce free per head
                nc.vector.tensor_mul(out=gd[:], in0=gd[:], in1=IT16[:, :1024])
                nc.vector.tensor_reduce(
                    out=S2c[:, dg * 8:(dg + 1) * 8],
                    in_=gd[:].rearrange("p (i e) -> p i e", i=8),
                    axis=mybir.AxisListType.X,
                    op=mybir.AluOpType.add,
                )

            # ---- stage local stats and AllReduce ----
            nc.sync.dma_start(out=stats_in[0:1, :], in_=s1row[0:1, :])
            nc.sync.dma_start(
                out=stats_in[1:2, :].rearrange("o (i d) -> o d i", i=H),
                in_=S2c[:],
            )
            nc.gpsimd.collective_compute(
                "AllReduce",
                mybir.AluOpType.add,
                replica_groups=groups,
                ins=[stats_in[:]],
                outs=[stats_out[:]],
            )
            nc.sync.dma_start(
                out=S1c[:], in_=stats_out[0:1, :].rearrange("o (i d) -> o d i", i=H)
            )
            nc.sync.dma_start(
                out=S2c[:], in_=stats_out[1:2, :].rearrange("o (i d) -> o d i", i=H)
            )

            # ---- stats math in [128(d), 16(i)] layout ----
            nc.vector.tensor_scalar_mul(mu[:], S1c[:], 1.0 / N)
            nc.vector.tensor_mul(out=m2[:], in0=mu[:], in1=mu[:])
            nc.vector.tensor_scalar_mul(m2[:], m2[:], -float(N))
            nc.vector.tensor_add(out=var[:], in0=S2c[:], in1=m2[:])
            nc.vector.tensor_scalar_mul(var[:], var[:], 1.0 / (N - 1))
            nc.scalar.sqrt(sig[:], var[:])
            nc.vector.tensor_scalar_add(sig[:], sig[:], EPS)
            nc.vector.reciprocal(recip[:], sig[:])                 # 1/(sigma+eps)
            nc.vector.tensor_scalar_mul(r_col[:], recip[:], 1.0 / math.sqrt(N))
            nc.vector.tensor_mul(out=q_col[:], in0=mu[:], in1=recip[:])
            nc.vector.tensor_scalar_mul(q8_col[:], q_col[:], 1.0 / NC)

            # ---- stage r,q rows and broadcast down partitions ----
            nc.sync.dma_start(
                out=rq_stage[0:1, :].rearrange("o (i d) -> o d i", i=H), in_=r_col[:]
            )
            nc.sync.dma_start(
                out=rq_stage[1:2, :].rearrange("o (i d) -> o d i", i=H), in_=q_col[:]
            )
            nc.sync.dma_start(out=R2[:], in_=rq_stage[0:1, :].broadcast_to([128, F]))
            nc.sync.dma_start(out=Q2[:], in_=rq_stage[1:2, :].broadcast_to([128, F]))

            # ---- pair gens: raw gram blocks, correction, DMA to bounce ----
            for g in range(NGEN):
                ps = psp.tile([128, 1024], F32, tag="ps")
                pr = [PAIRS[8 * g + c] for c in range(8)]
                # maximal merged runs: same i, consecutive j, within a PSUM bank half
                runs = []
                for half in (0, 4):
                    s0 = half
                    while s0 < half + 4:
                        i0, j0 = pr[s0]
                        e0 = s0 + 1
                        while (
                            e0 < half + 4
                            and pr[e0][0] == i0
                            and pr[e0][1] == pr[e0 - 1][1] + 1
                        ):
                            e0 += 1
                        runs.append((s0, e0, i0, j0))
                        s0 = e0
                for k in range(KCH):
                    for (s0, e0, i0, j0) in runs:
                        nb = e0 - s0
                        nc.tensor.matmul(
                            ps[:, s0 * DH:e0 * DH],
                            lhsT=xb[k][:, i0 * DH:(i0 + 1) * DH],
                            rhs=xb[k][:, j0 * DH:(j0 + nb) * DH],
                            start=(k == 0),
                            stop=(k == KCH - 1),
                        )
                gs = gsbp.tile([128, 1024], F32, tag="gsb")
                nc.scalar.copy(out=gs[:], in_=ps[:])
                vt = postp.tile([128, 1024], F32, tag="vt")
                for c in range(8):
                    i, j = PAIRS[8 * g + c]
                    u = gs[:, c * DH:(c + 1) * DH]
                    nc.vector.tensor_mul(out=u, in0=u, in1=R2[:, j * DH:(j + 1) * DH])
                    nc.vector.tensor_scalar_mul(u, u, r_col[:, i:i + 1])
                    v = vt[:, c * DH:(c + 1) * DH]
                    nc.vector.tensor_scalar_mul(
                        v, Q2[:, j * DH:(j + 1) * DH], q8_col[:, i:i + 1]
                    )
                    nc.vector.tensor_sub(out=u, in0=u, in1=v)
                t = g // 4 if g < 12 else 3
                lo, hi = RS_SLOTS[t]
                for c in range(8):
                    nc.sync.dma_start(
                        out=bounce[t][c, g - lo], in_=gs[:, c * DH:(c + 1) * DH]
                    )
                # issue the ReduceScatter as soon as its last gen is written
                if g == min(lo + (hi - lo) - 1, NGEN - 1) and g in (3, 7, 11, 14):
                    nc.gpsimd.collective_compute(
                        "ReduceScatter",
                        mybir.AluOpType.add,
                        replica_groups=groups,
                        ins=[bounce[t][:]],
                        outs=[rsout[t][:]],
                    )
                    for b in range(hi - lo):
                        rb = postp.tile([128, DH], F32, tag="rb")
                        nc.sync.dma_start(out=rb[:], in_=rsout[t][b])
                        nc.vector.tensor_sub(out=rb[:], in0=rb[:], in1=IT[:])
                        nc.scalar.activation(
                            out=rb[:],
                            in_=rb[:],
                            func=mybir.ActivationFunctionType.Square,
                            accum_out=pl_cols[:, lo + b:lo + b + 1],
                        )

            # ---- partition-reduce pair losses and write out ----
            pspl = psp.tile([128, 1024], F32, tag="ps")
            nc.tensor.matmul(
                pspl[0:1, 0:NGEN], lhsT=onesf[:], rhs=pl_cols[:], start=True, stop=True
            )
            outsb = sg.tile([1, NGEN], F32, tag="outsb")
            nc.vector.tensor_copy(out=outsb[:], in_=pspl[0:1, 0:NGEN])
            nc.sync.dma_start(out=out[:], in_=outsb[:])

    nc.compile()
    return nc


_NC_CACHE = None


def _get_nc():
    global _NC_CACHE
    if _NC_CACHE is None:
        _NC_CACHE = build()
    return _NC_CACHE


def _make_in_maps(head_outputs):
    shards = np.asarray(head_outputs, dtype=np.float32).reshape(NC, NS, F)
    return [{"x": np.ascontiguousarray(shards[c])} for c in range(NC)]


def _combine(results, G):
    pl = np.zeros(len(PAIRS), np.float64)
    for c in range(NC):
        o = np.asarray(results[c]["out"], dtype=np.float64).reshape(NGEN)
        for b in range(NGEN):
            pl[8 * b + c] = o[b]
    Gd = np.asarray(G, dtype=np.float64)
    w = ALPHA + (1.0 - ALPHA) * np.logaddexp(0.0, -BETA * (Gd - TAU))
    loss = sum(w[i, j] * pl[p] for p, (i, j) in enumerate(PAIRS)) / len(PAIRS)
    return np.asarray(loss, dtype=np.float32)


def kernel(head_outputs, G):
    nc = _get_nc()
    res = run_bass_kernel_spmd(nc, _make_in_maps(head_outputs), list(range(NC)))
    return _combine(res.results, G)


def timed_run(head_outputs, G, **kw):
    """Run with NTFF profiling; returns (loss, BassKernelResults)."""
    nc = _get_nc()
    res = run_bass_kernel_spmd(
        nc, _make_in_maps(head_outputs), list(range(NC)), trace=True, **kw
    )
    return _combine(res.results, G), res


# revision 11
# speedup vs baseline: 1.1359x; 1.1359x over previous
"""AdaptiveBarlowTwinsLoss on 8 TRN2 NeuronCores.

Math: with O = head_outputs reshaped (N, H*dh), standardized O~ = (O-mu)/(sigma+eps),
the loss only needs the 120 upper-triangular head-pair blocks of C = O~^T O~ / N.
Writing G = O^T O (raw gram), C[id, je] = G[id,je]*r[id]*r[je] - q[id]*q[je]
with r = 1/(sqrt(N)(sigma+eps)), q = sqrt(N)*mu*r.  So each core computes the raw
gram over its token shard (bf16 matmuls, fp32 PSUM accum), applies the affine
standardization correction locally (scaled by 1/8 for the rank-1 term), and a
ReduceScatter sums the corrected blocks while scattering 15 pairs to each core.
Each core then computes ||C_ij - I||_F^2 for its pairs; the host applies the
(16,16) softplus pair weights and averages.

Pair p (lexicographic (i,j), i<j) is assigned to core p % 8, slot p // 8.
"""

import math
import sys

sys.path.insert(0, "/opt/trn_rl_repo")

import numpy as np

import concourse.bass as bass
import concourse.tile as tile
from concourse import bacc, mybir
from concourse.bass_utils import run_bass_kernel_spmd
from concourse.masks import make_identity

B, T, H, DH = 8, 2048, 16, 128
N = B * T                      # 16384 tokens
F = H * DH                     # 2048 features
NC = 8                         # cores
NS = N // NC                   # 2048 tokens per core
KCH = NS // 128                # 16 contraction chunks of 128 tokens
ALPHA, BETA, TAU, EPS = 0.929, 15.99, 0.0, 1e-8

PAIRS = [(i, j) for i in range(H) for j in range(i + 1, H)]   # 120, lex order
NGEN = len(PAIRS) // 8                                        # 15 gens of 8 blocks
RS_SLOTS = [(0, 4), (4, 8), (8, 12), (12, 15)]                # 4 ReduceScatter calls

F32 = mybir.dt.float32
BF16 = mybir.dt.bfloat16


def build():
    nc = bacc.Bacc("TRN2", target_bir_lowering=False, debug=False, num_devices=NC)

    x = nc.dram_tensor("x", [NS, F], F32, kind="ExternalInput")
    out = nc.dram_tensor("out", [1, NGEN], F32, kind="ExternalOutput")
    groups = [list(range(NC))]

    with tile.TileContext(nc) as tc:
        with (
            tc.tile_pool(name="dram", bufs=1, space="DRAM") as dram,
            tc.tile_pool(name="xf", bufs=3) as xfp,
            tc.tile_pool(name="xb", bufs=1) as xbp,
            tc.tile_pool(name="gsb", bufs=6) as gsbp,
            tc.tile_pool(name="ps", bufs=3, space="PSUM") as psp,
            tc.tile_pool(name="sg", bufs=1) as sg,
            tc.tile_pool(name="post", bufs=2) as postp,
        ):
            # ---- DRAM internals ----
            stats_in = dram.tile([2, F], F32, tag="stats_in")
            stats_out = dram.tile([2, F], F32, tag="stats_out")
            rq_stage = dram.tile([2, F], F32, tag="rq_stage")
            bounce = [
                dram.tile([NC, hi - lo, DH, DH], BF16, tag=f"bounce{t}", name=f"bounce{t}")
                for t, (lo, hi) in enumerate(RS_SLOTS)
            ]
            rsout = [
                dram.tile([hi - lo, DH, DH], BF16, tag=f"rsout{t}", name=f"rsout{t}")
                for t, (lo, hi) in enumerate(RS_SLOTS)
            ]

            # ---- persistent SBUF ----
            xb = [
                xbp.tile([128, F], BF16, tag=f"xb{k}", name=f"xb{k}")
                for k in range(KCH)
            ]
            s1acc = sg.tile([128, F], F32, tag="s1acc")
            IT16 = sg.tile([128, F], F32, tag="it16")     # identity per head block
            ITb = sg.tile([128, DH], BF16, tag="itb")
            onesf = sg.tile([128, 1], F32, tag="ones")
            R2 = sg.tile([128, F], F32, tag="r2")         # r broadcast down partitions
            Q2 = sg.tile([128, F], F32, tag="q2")         # q broadcast down partitions
            pl_cols = sg.tile([128, NGEN], F32, tag="plc")

            def colt(tag):
                return sg.tile([128, H], F32, tag=tag, name=tag)

            S1c, S2c = colt("s1c"), colt("s2c")
            mu, m2, var = colt("mu"), colt("m2"), colt("var")
            sig, recip = colt("sig"), colt("recip")
            r_col, q_col, q8_col = colt("rcol"), colt("qcol"), colt("q8col")

            # ---- constants ----
            nc.gpsimd.memset(onesf[:], 1.0)
            make_identity(nc, ITb[:])
            nc.gpsimd.memset(IT16[:], 0.0)
            it3 = IT16[:].rearrange("p (i e) -> p i e", i=H)
            nc.gpsimd.affine_select(
                out=it3,
                in_=it3,
                compare_op=mybir.AluOpType.not_equal,
                fill=1.0,
                base=0,
                pattern=[[0, H], [-1, DH]],
                channel_multiplier=1,
            )

            # ---- load, cast to bf16, accumulate S1 ----
            for k in range(KCH):
                xf = xfp.tile([128, F], F32, tag="xf")
                nc.sync.dma_start(out=xf[:], in_=x[k * 128:(k + 1) * 128, :])
                nc.scalar.copy(out=xb[k][:], in_=xf[:])
                if k == 0:
                    nc.vector.tensor_copy(out=s1acc[:], in_=xf[:])
                else:
                    nc.vector.tensor_add(out=s1acc[:], in0=s1acc[:], in1=xf[:])

            # S1 partition reduce via ones-matmul -> s1row [1, F]
            s1row = sg.tile([1, F], F32, tag="s1row")
            for t4 in range(4):
                pss1 = psp.tile([1, 512], F32, tag="pss1", bufs=2)
                nc.tensor.matmul(
                    pss1[0:1, :],
                    lhsT=onesf[:],
                    rhs=s1acc[:, t4 * 512:(t4 + 1) * 512],
                    start=True,
                    stop=True,
                )
                nc.vector.tensor_copy(
                    out=s1row[0:1, t4 * 512:(t4 + 1) * 512], in_=pss1[0:1, :]
                )

            # ---- diagonal blocks G_ii (for S2 = diag) : 2 gens of 8 ----
            for dg in range(2):
                ps = psp.tile([128, 1024], F32, tag="ps")
                for k in range(KCH):
                    for b in range(8):
                        i = dg * 8 + b
                        sl = xb[k][:, i * DH:(i + 1) * DH]
                        nc.tensor.matmul(
                            ps[:, b * DH:(b + 1) * DH],
                            lhsT=sl,
                            rhs=sl,
                            start=(k == 0),
                            stop=(k == KCH - 1),
                        )
                gd = postp.tile([128, 1024], F32, tag="gd")
                nc.scalar.copy(out=gd[:], in_=ps[:])
                # extract diagonals: multiply by tiled identity, reduce free per head
                nc.vector.tensor_mul(out=gd[:], in0=gd[:], in1=IT16[:, :1024])
                nc.vector.tensor_reduce(
                    out=S2c[:, dg * 8:(dg + 1) * 8],
                    in_=gd[:].rearrange("p (i e) -> p i e", i=8),
                    axis=mybir.AxisListType.X,
                    op=mybir.AluOpType.add,
                )

            # ---- stage local stats and AllReduce ----
            nc.sync.dma_start(out=stats_in[0:1, :], in_=s1row[0:1, :])
            nc.sync.dma_start(
                out=stats_in[1:2, :].rearrange("o (i d) -> o d i", i=H),
                in_=S2c[:],
            )
            nc.gpsimd.collective_compute(
                "AllReduce",
                mybir.AluOpType.add,
                replica_groups=groups,
                ins=[stats_in[:]],
                outs=[stats_out[:]],
            )
            nc.sync.dma_start(
                out=S1c[:], in_=stats_out[0:1, :].rearrange("o (i d) -> o d i", i=H)
            )
            nc.sync.dma_start(
                out=S2c[:], in_=stats_out[1:2, :].rearrange("o (i d) -> o d i", i=H)
            )

            # ---- stats math in [128(d), 16(i)] layout ----
            nc.vector.tensor_scalar_mul(mu[:], S1c[:], 1.0 / N)
            nc.vector.tensor_mul(out=m2[:], in0=mu[:], in1=mu[:])
            nc.vector.tensor_scalar_mul(m2[:], m2[:], -float(N))
            nc.vector.tensor_add(out=var[:], in0=S2c[:], in1=m2[:])
            nc.vector.tensor_scalar_mul(var[:], var[:], 1.0 / (N - 1))
            nc.scalar.sqrt(sig[:], var[:])
            nc.vector.tensor_scalar_add(sig[:], sig[:], EPS)
            nc.vector.reciprocal(recip[:], sig[:])                 # 1/(sigma+eps)
            nc.vector.tensor_scalar_mul(r_col[:], recip[:], 1.0 / math.sqrt(N))
            nc.vector.tensor_mul(out=q_col[:], in0=mu[:], in1=recip[:])
            nc.vector.tensor_scalar_mul(q8_col[:], q_col[:], 1.0 / NC)

            # ---- stage r,q rows and broadcast down partitions ----
            nc.sync.dma_start(
                out=rq_stage[0:1, :].rearrange("o (i d) -> o d i", i=H), in_=r_col[:]
            )
            nc.sync.dma_start(
                out=rq_stage[1:2, :].rearrange("o (i d) -> o d i", i=H), in_=q_col[:]
            )
            nc.sync.dma_start(out=R2[:], in_=rq_stage[0:1, :].broadcast_to([128, F]))
            nc.sync.dma_start(out=Q2[:], in_=rq_stage[1:2, :].broadcast_to([128, F]))

            # ---- pair gens: row segments (same i, consecutive j, nb<=8) ----
            # Raw gram blocks accumulate in PSUM; corrections are batched
            # per-segment: A = G*(r_i (x) r_j) - q_i (x) q_j / 8, output bf16.
            segs = []
            p0 = 0
            for i in range(H):
                j = i + 1
                while j < H:
                    nb = min(8, H - j)
                    segs.append((i, j, nb, p0))
                    p0 += nb
                    j += nb
            assert p0 == len(PAIRS)
            rs_trigger = {8 * hi - 1: t for t, (lo, hi) in enumerate(RS_SLOTS)}

            for (i, j0, nb, pbase) in segs:
                w = nb * DH
                ps = psp.tile([128, 1024], F32, tag="ps", name="ps")
                for k in range(KCH):
                    for c0 in range(0, w, 512):
                        c1 = min(c0 + 512, w)
                        nc.tensor.matmul(
                            ps[:, c0:c1],
                            lhsT=xb[k][:, i * DH:(i + 1) * DH],
                            rhs=xb[k][:, j0 * DH + c0:j0 * DH + c1],
                            start=(k == 0),
                            stop=(k == KCH - 1),
                        )
                gs = gsbp.tile([128, 1024], F32, tag="gsb", name="gs")
                nc.scalar.copy(out=gs[:, :w], in_=ps[:, :w])
                # RIJ = r_i (x) r_j slice; V = q_i (x) q_j / 8 — both on ACT
                rij = postp.tile([128, 1024], F32, tag="rij", name="rij")
                nc.scalar.activation(
                    out=rij[:, :w],
                    in_=R2[:, j0 * DH:j0 * DH + w],
                    func=mybir.ActivationFunctionType.Copy,
                    scale=r_col[:, i:i + 1],
                )
                vt = postp.tile([128, 1024], F32, tag="vt", name="vt")
                nc.scalar.activation(
                    out=vt[:, :w],
                    in_=Q2[:, j0 * DH:j0 * DH + w],
                    func=mybir.ActivationFunctionType.Copy,
                    scale=q8_col[:, i:i + 1],
                )
                nc.vector.tensor_mul(out=gs[:, :w], in0=gs[:, :w], in1=rij[:, :w])
                ab = gsbp.tile([128, 1024], BF16, tag="ab", name="ab")
                nc.vector.tensor_sub(out=ab[:, :w], in0=gs[:, :w], in1=vt[:, :w])
                for m in range(nb):
                    p = pbase + m
                    c, b = p % 8, p // 8
                    t = min(b // 4, 3)
                    lo, hi = RS_SLOTS[t]
                    nc.sync.dma_start(
                        out=bounce[t][c, b - lo], in_=ab[:, m * DH:(m + 1) * DH]
                    )
                # issue each ReduceScatter as soon as its last block is written
                for p in range(pbase, pbase + nb):
                    if p in rs_trigger:
                        t = rs_trigger[p]
                        lo, hi = RS_SLOTS[t]
                        nc.gpsimd.collective_compute(
                            "ReduceScatter",
                            mybir.AluOpType.add,
                            replica_groups=groups,
                            ins=[bounce[t][:]],
                            outs=[rsout[t][:]],
                        )
                        for b in range(hi - lo):
                            rb = postp.tile([128, DH], BF16, tag="rb", name="rb")
                            nc.sync.dma_start(out=rb[:], in_=rsout[t][b])
                            rbf = postp.tile([128, DH], F32, tag="rbf", name="rbf")
                            nc.vector.tensor_sub(out=rbf[:], in0=rb[:], in1=ITb[:])
                            nc.scalar.activation(
                                out=rbf[:],
                                in_=rbf[:],
                                func=mybir.ActivationFunctionType.Square,
                                accum_out=pl_cols[:, lo + b:lo + b + 1],
                            )

            # ---- partition-reduce pair losses and write out ----
            pspl = psp.tile([128, 1024], F32, tag="ps")
            nc.tensor.matmul(
                pspl[0:1, 0:NGEN], lhsT=onesf[:], rhs=pl_cols[:], start=True, stop=True
            )
            outsb = sg.tile([1, NGEN], F32, tag="outsb")
            nc.vector.tensor_copy(out=outsb[:], in_=pspl[0:1, 0:NGEN])
            nc.sync.dma_start(out=out[:], in_=outsb[:])

    nc.compile()
    return nc


_NC_CACHE = None


def _get_nc():
    global _NC_CACHE
    if _NC_CACHE is None:
        _NC_CACHE = build()
    return _NC_CACHE


def _make_in_maps(head_outputs):
    shards = np.asarray(head_outputs, dtype=np.float32).reshape(NC, NS, F)
    return [{"x": np.ascontiguousarray(shards[c])} for c in range(NC)]


def _combine(results, G):
    pl = np.zeros(len(PAIRS), np.float64)
    for c in range(NC):
        o = np.asarray(results[c]["out"], dtype=np.float64).reshape(NGEN)
        for b in range(NGEN):
            pl[8 * b + c] = o[b]
    Gd = np.asarray(G, dtype=np.float64)
    w = ALPHA + (1.0 - ALPHA) * np.logaddexp(0.0, -BETA * (Gd - TAU))
    loss = sum(w[i, j] * pl[p] for p, (i, j) in enumerate(PAIRS)) / len(PAIRS)
    return np.asarray(loss, dtype=np.float32)


def kernel(head_outputs, G):
    nc = _get_nc()
    res = run_bass_kernel_spmd(nc, _make_in_maps(head_outputs), list(range(NC)))
    return _combine(res.results, G)


def timed_run(head_outputs, G, **kw):
    """Run with NTFF profiling; returns (loss, BassKernelResults)."""
    nc = _get_nc()
    res = run_bass_kernel_spmd(
        nc, _make_in_maps(head_outputs), list(range(NC)), trace=True, **kw
    )
    return _combine(res.results, G), res
